# revision 20
# baseline (speedup 1.0000x reference)
"""Trainium2 Bass kernel for nn_CentralAttentiveModule.

Math (see reference):
    v = x@Wv.T+bv ; k = x@Wk.T(+bk, cancels in softmax) ; q = x@Wq.T(+bq)
    qseg = segment_max(q) ; M = sum(qseg[cluster]*k, -1)
    attn = segment_softmax(M) ; h = attn[:,None]*v
    out = relu(batchnorm(h))

Layout: points sorted by cluster on host; clusters size-sorted and dealt
round-robin to 16 strips (8 cores x 2 partition halves; feature-major:
partition = feature x strip, free = slot).  Each cluster's points are
padded to one fixed window of ceil(cnt/8)*8 slots, so every segment
max / sum / broadcast is a chunk-local fixed-window vector op -- no
cross-chunk combine, no gathers.  All strips share one region schedule
(per window-size counts maxed over strips; shortfall windows get a
single unmasked zero "fake" slot so den=1, corrected in BN stats).
Matmuls in bf16 (single-pass PE).  Pad slots are masked to -1e30 before
exp via a tiny K=2 matmul accumulated into the M matmul.  BN stats
AllReduced across the 8 cores in-kernel.
"""
import numpy as np
import ml_dtypes

import concourse.bacc as bacc
import concourse.tile as tile
from concourse import mybir
from concourse.bass_utils import run_bass_kernel_spmd

N_TOT = 500_000
D = 64
C_TOT = 10_000
NCORES = 8
NSTRIPS = 16
GRID = 8
LMAX = 512
BN_EPS = 1e-5
BIGNEG = -1.0e30
F32 = mybir.dt.float32
F16 = mybir.dt.float16
BF16 = mybir.dt.bfloat16
BF = ml_dtypes.bfloat16


# ----------------------------------------------------------------- host prep
def _host_prep(cluster):
    counts = np.bincount(cluster, minlength=C_TOT)
    order = np.argsort(cluster, kind="stable")
    pt_start = np.concatenate([[0], np.cumsum(counts)])
    wb = np.maximum((counts + GRID - 1) // GRID, 1) * GRID
    assert int(wb.max()) <= LMAX

    rank = np.argsort(-wb, kind="stable")
    strips = [rank[s::NSTRIPS] for s in range(NSTRIPS)]

    vals = sorted(set(wb.tolist()), reverse=True)
    prof = {v: max(int((wb[st] == v).sum()) for st in strips) for v in vals}

    # region schedule shared by every strip/core: (slot_off, L, v, nwin)
    schedule = []
    off = 0
    for v in vals:
        total = prof[v] * v
        lmax = (LMAX // v) * v
        o = 0
        while o < total:
            L = min(lmax, total - o)
            schedule.append((off + o, L, v, L // v))
            o += L
        off += total
    W = off
    return dict(counts=counts, order=order, pt_start=pt_start, wb=wb,
                strips=strips, vals=vals, prof=prof, schedule=schedule, W=W)


def _strip_layout(prep, s):
    """slots/pts mapping + padflag + fake count for strip s."""
    counts, order, pt_start = prep["counts"], prep["order"], prep["pt_start"]
    wb, vals, prof, W = prep["wb"], prep["vals"], prep["prof"], prep["W"]
    cl = prep["strips"][s]

    padflag = np.ones(W, np.float32)
    slot_list, pt_list = [], []
    nfake = 0
    off = 0
    for v in vals:
        mine = cl[wb[cl] == v]
        for w in range(prof[v]):
            ws = off + w * v
            if w < len(mine) and counts[mine[w]] > 0:
                c = mine[w]
                cnt = int(counts[c])
                slot_list.append(np.arange(ws, ws + cnt))
                pt_list.append(order[pt_start[c]: pt_start[c] + cnt])
                padflag[ws: ws + cnt] = 0.0
            else:
                padflag[ws] = 0.0  # fake slot: x=0 -> e=1, den=1, ht=bv
                nfake += 1
        off += prof[v] * v
    slots = (np.concatenate(slot_list) if slot_list else np.zeros(0, np.int64))
    pts = (np.concatenate(pt_list) if pt_list else np.zeros(0, np.int64))
    return slots, pts, padflag, nfake


# ------------------------------------------------------------- build program
def _build_program(W, schedule):
    nchunks = len(schedule)
    nc = bacc.Bacc("TRN2", target_bir_lowering=False, debug=False,
                   num_devices=NCORES)

    def din(name, shape, dt=F32):
        return nc.dram_tensor(name, shape, dt, kind="ExternalInput")

    xin = din("xin", [128, W], BF16)
    pflag = din("pflag", [2, W], BF16)
    wqt = din("wqt", [128, 64], BF16)
    wkt = din("wkt", [128, 64], BF16)
    wvt = din("wvt", [128, 64], BF16)
    e2big = din("e2big", [128, 128], BF16)
    maskq = din("maskq", [2, 128], BF16)
    bq2 = din("bq2", [128, 1])
    bv2 = din("bv2", [128, 1])
    gamma2 = din("gamma2", [128, 1])
    beta2 = din("beta2", [128, 1])
    fakecorr = din("fakecorr", [128, 2])
    hout = nc.dram_tensor("hout", [128, W], F32, kind="ExternalOutput")

    MM = dict(skip_group_check=True)

    with tile.TileContext(nc, pool_alloc_mode="queue") as tc:
        with tc.tile_pool(name="const", bufs=1) as cpool, \
             tc.tile_pool(name="p2x", bufs=4) as p2x, \
             tc.tile_pool(name="scr", bufs=4) as scr, \
             tc.tile_pool(name="small", bufs=4) as small, \
             tc.tile_pool(name="htp", bufs=nchunks + 1) as htp, \
             tc.tile_pool(name="sums", bufs=1) as sums, \
             tc.tile_pool(name="ps", bufs=2, space="PSUM") as ps, \
             tc.tile_pool(name="dram", bufs=2, space="DRAM") as dram:
            c_wqt = cpool.tile([128, 64], BF16)
            nc.scalar.dma_start(c_wqt[:], wqt[:])
            c_wkt = cpool.tile([128, 64], BF16)
            nc.scalar.dma_start(c_wkt[:], wkt[:])
            c_wvt = cpool.tile([128, 64], BF16)
            nc.scalar.dma_start(c_wvt[:], wvt[:])
            c_e2big = cpool.tile([128, 128], BF16)
            nc.gpsimd.dma_start(c_e2big[:], e2big[:])
            c_maskq = cpool.tile([2, 128], BF16)
            nc.gpsimd.dma_start(c_maskq[:], maskq[:])
            c_bq2 = cpool.tile([128, 1], F32)
            nc.gpsimd.dma_start(c_bq2[:], bq2[:])
            c_bv2 = cpool.tile([128, 1], F32)
            nc.scalar.dma_start(c_bv2[:], bv2[:])

            sumh = sums.tile([128, nchunks], F32)
            sumsq = sums.tile([128, nchunks], F32)

            # warmup collective: absorb ring-setup latency during streaming
            win = dram.tile([128, 2], F32, tag="win")
            wout = dram.tile([128, 2], F32, tag="wout")
            warm = sums.tile([128, 2], F32)
            nc.vector.memset(warm[:], 0.0)
            nc.gpsimd.dma_start(win[:], warm[:])
            nc.gpsimd.collective_compute(
                "AllReduce", mybir.AluOpType.add,
                replica_groups=[list(range(NCORES))],
                ins=[win.opt()], outs=[wout.opt()])

            state = [None] * nchunks  # (vp, et, ht, j) skew carry

            def stage_a(j):
                off, L, v, nw = schedule[j]
                sl = slice(off, off + L)
                xt = p2x.tile([128, LMAX], BF16, tag="xt")
                nc.sync.dma_start(xt[:, :L], xin[:, sl])
                pf = p2x.tile([2, LMAX], BF16, tag="pf")
                nc.sync.dma_start(pf[:, :L], pflag[:, sl])
                qp = ps.tile([128, LMAX], F32, space="PSUM", tag="qp")
                nc.tensor.matmul(out=qp[0:64, :L], lhsT=c_wqt[0:64, :],
                                 rhs=xt[0:64, :L], start=True, stop=False,
                                 tile_position=(0, 0), **MM)
                nc.tensor.matmul(out=qp[64:128, :L], lhsT=c_wqt[64:128, :],
                                 rhs=xt[64:128, :L], start=True, stop=True,
                                 tile_position=(64, 64), **MM)
                kp = ps.tile([128, LMAX], F32, space="PSUM", tag="kp")
                nc.tensor.matmul(out=kp[0:64, :L], lhsT=c_wkt[0:64, :],
                                 rhs=xt[0:64, :L], start=True, stop=False,
                                 tile_position=(0, 0), **MM)
                nc.tensor.matmul(out=kp[64:128, :L], lhsT=c_wkt[64:128, :],
                                 rhs=xt[64:128, :L], start=True, stop=True,
                                 tile_position=(64, 64), **MM)
                vp = ps.tile([128, LMAX], F32, space="PSUM", tag="vp")
                nc.tensor.matmul(out=vp[0:64, :L], lhsT=c_wvt[0:64, :],
                                 rhs=xt[0:64, :L], start=True, stop=False,
                                 tile_position=(0, 0), **MM)
                nc.tensor.matmul(out=vp[64:128, :L], lhsT=c_wvt[64:128, :],
                                 rhs=xt[64:128, :L], start=True, stop=True,
                                 tile_position=(64, 64), **MM)
                # window max of q + bias -> per-window query, broadcast via AP
                qs = small.tile([128, 64], F32, tag="qs")
                nc.vector.tensor_reduce(
                    out=qs[:, :nw],
                    in_=qp[:, :L].rearrange("p (n l) -> p n l", l=v),
                    axis=mybir.AxisListType.X, op=mybir.AluOpType.max)
                pt = scr.tile([128, LMAX], BF16, tag="pt")
                nc.vector.scalar_tensor_tensor(
                    out=pt[:, :L].rearrange("p (n l) -> p n l", l=v),
                    in0=qs[:, :nw].to_broadcast([128, nw, v]),
                    scalar=c_bq2[:],
                    in1=kp[:, :L].rearrange("p (n l) -> p n l", l=v),
                    op0=mybir.AluOpType.add, op1=mybir.AluOpType.mult)
                mp = ps.tile([128, LMAX], F32, space="PSUM", tag="mp")
                nc.tensor.matmul(out=mp[:, :L], lhsT=c_e2big[:], rhs=pt[:, :L],
                                 start=True, stop=False, **MM)
                nc.tensor.matmul(out=mp[:, :L], lhsT=c_maskq[:], rhs=pf[:, :L],
                                 start=False, stop=True, **MM)
                et = scr.tile([128, LMAX], BF16, tag="et")
                nc.scalar.activation(out=et[:, :L], in_=mp[:, :L],
                                     func=mybir.ActivationFunctionType.Exp)
                state[j] = (vp, et)

            def stage_b(j):
                off, L, v, nw = schedule[j]
                vp, et = state[j]
                dn = small.tile([128, 64], F32, tag="dn")
                nc.vector.tensor_reduce(
                    out=dn[:, :nw],
                    in_=et[:, :L].rearrange("p (n l) -> p n l", l=v),
                    axis=mybir.AxisListType.X, op=mybir.AluOpType.add)
                iv = small.tile([128, 64], F32, tag="iv")
                nc.vector.reciprocal(out=iv[:, :nw], in_=dn[:, :nw])
                at = scr.tile([128, LMAX], F32, tag="at")
                nc.vector.tensor_tensor(
                    out=at[:, :L].rearrange("p (n l) -> p n l", l=v),
                    in0=iv[:, :nw].to_broadcast([128, nw, v]),
                    in1=et[:, :L].rearrange("p (n l) -> p n l", l=v),
                    op=mybir.AluOpType.mult)
                ht = htp.tile([128, LMAX], F16, tag="ht")
                nc.vector.scalar_tensor_tensor(
                    out=ht[:, :L], in0=vp[:, :L], scalar=c_bv2[:],
                    in1=at[:, :L], op0=mybir.AluOpType.add,
                    op1=mybir.AluOpType.mult, accum_out=sumh[:, j:j + 1])
                sq = scr.tile([128, LMAX], F32, tag="sq")
                nc.scalar.activation(out=sq[:, :L], in_=ht[:, :L],
                                     func=mybir.ActivationFunctionType.Square,
                                     accum_out=sumsq[:, j:j + 1])
                state[j] = ht

            stage_a(0)
            for j in range(1, nchunks):
                stage_a(j)
                stage_b(j - 1)
            stage_b(nchunks - 1)

            # BN stats: fold chunks, fake-slot fix, fold strips, AllReduce
            st = sums.tile([128, 2], F32)
            nc.vector.tensor_reduce(out=st[:, 0:1], in_=sumh[:],
                                    axis=mybir.AxisListType.X,
                                    op=mybir.AluOpType.add)
            nc.vector.tensor_reduce(out=st[:, 1:2], in_=sumsq[:],
                                    axis=mybir.AxisListType.X,
                                    op=mybir.AluOpType.add)
            c_fake = sums.tile([128, 2], F32)
            nc.sync.dma_start(c_fake[:], fakecorr[:])
            nc.vector.tensor_tensor(out=st[:], in0=st[:], in1=c_fake[:],
                                    op=mybir.AluOpType.subtract)
            stB = sums.tile([64, 2], F32)
            nc.sync.dma_start(stB[:], st[64:128, :])
            stAll = sums.tile([128, 2], F32)
            nc.vector.memset(stAll[:], 0.0)
            nc.vector.tensor_tensor(out=stAll[0:64, :], in0=st[0:64, :],
                                    in1=stB[:], op=mybir.AluOpType.add)
            cin = dram.tile([128, 2], F32, tag="cin")
            cout = dram.tile([128, 2], F32, tag="cout")
            nc.gpsimd.dma_start(cin[:], stAll[:])
            nc.gpsimd.collective_compute(
                "AllReduce", mybir.AluOpType.add,
                replica_groups=[list(range(NCORES))],
                ins=[cin.opt()], outs=[cout.opt()])
            glob = sums.tile([64, 2], F32)
            nc.sync.dma_start(glob[:], cout[0:64, :])

            mean = sums.tile([64, 1], F32)
            nc.vector.tensor_scalar_mul(out=mean[:], in0=glob[:, 0:1],
                                        scalar1=1.0 / N_TOT)
            ex2 = sums.tile([64, 1], F32)
            nc.vector.tensor_scalar_mul(out=ex2[:], in0=glob[:, 1:2],
                                        scalar1=1.0 / N_TOT)
            var = sums.tile([64, 1], F32)
            nc.vector.tensor_tensor(out=var[:], in0=mean[:], in1=mean[:],
                                    op=mybir.AluOpType.mult)
            nc.vector.tensor_tensor(out=var[:], in0=ex2[:], in1=var[:],
                                    op=mybir.AluOpType.subtract)
            nc.vector.tensor_scalar_add(out=var[:], in0=var[:], scalar1=BN_EPS)
            sd = sums.tile([64, 1], F32)
            nc.scalar.activation(out=sd[:], in_=var[:],
                                 func=mybir.ActivationFunctionType.Sqrt)
            nc.vector.reciprocal(out=sd[:], in_=sd[:])
            c_g2 = sums.tile([128, 1], F32)
            nc.sync.dma_start(c_g2[:], gamma2[:])
            c_b2 = sums.tile([128, 1], F32)
            nc.sync.dma_start(c_b2[:], beta2[:])
            ab = sums.tile([64, 2], F32)
            nc.vector.tensor_tensor(out=ab[:, 0:1], in0=c_g2[0:64, :], in1=sd[:],
                                    op=mybir.AluOpType.mult)
            nc.vector.tensor_tensor(out=ab[:, 1:2], in0=mean[:], in1=ab[:, 0:1],
                                    op=mybir.AluOpType.mult)
            nc.vector.tensor_tensor(out=ab[:, 1:2], in0=c_b2[0:64, :],
                                    in1=ab[:, 1:2], op=mybir.AluOpType.subtract)
            ab2 = sums.tile([128, 2], F32)
            nc.sync.dma_start(ab2[0:64, :], ab[:])
            nc.sync.dma_start(ab2[64:128, :], ab[:])

            # pass 4: out = relu(A*h + B)
            GR = 8
            with tc.tile_pool(name="otp", bufs=3) as otp:
                for g in range(0, nchunks, GR):
                    grp = schedule[g:g + GR]
                    goff = grp[0][0]
                    span = sum(c[1] for c in grp)
                    ot = otp.tile([128, GR * LMAX], F32, tag="ot")
                    for idx, (off, L, v, nw) in enumerate(grp):
                        ht = state[g + idx]
                        co = off - goff
                        if idx % 2 == 0:
                            nc.scalar.activation(
                                out=ot[:, co:co + L], in_=ht[:, :L],
                                func=mybir.ActivationFunctionType.Relu,
                                scale=ab2[:, 0:1], bias=ab2[:, 1:2])
                        else:
                            nc.vector.tensor_scalar(
                                out=ot[:, co:co + L], in0=ht[:, :L],
                                scalar1=ab2[:, 0:1], scalar2=ab2[:, 1:2],
                                op0=mybir.AluOpType.mult,
                                op1=mybir.AluOpType.add)
                            nc.vector.tensor_scalar_max(
                                out=ot[:, co:co + L], in0=ot[:, co:co + L],
                                scalar1=0.0)
                    if (g // GR) % 2 == 0:
                        nc.sync.dma_start(hout[:, goff:goff + span],
                                          ot[:, :span])
                    else:
                        nc.gpsimd.dma_start(hout[:, goff:goff + span],
                                            ot[:, :span])

    nc.compile()
    return nc


# ------------------------------------------------------------------- kernel
_CACHE = {}


def _prepare(pos, x, cluster, Wv, bv, Wk, bk, Wq, bq, gamma, beta):
    x = np.ascontiguousarray(np.asarray(x, np.float32))
    cluster = np.asarray(cluster).astype(np.int64)

    prep = _host_prep(cluster)
    W, schedule = prep["W"], prep["schedule"]

    key = (W, tuple(schedule))
    if key not in _CACHE:
        _CACHE[key] = _build_program(W, schedule)
    nc = _CACHE[key]

    maskq = np.zeros((2, 128), np.float32)
    maskq[0, 0:64] = BIGNEG
    maskq[1, 64:128] = BIGNEG
    e2big = np.zeros((128, 128), np.float32)
    e2big[0:64, 0:64] = 1.0
    e2big[64:128, 64:128] = 1.0
    bvf = np.asarray(bv, np.float32)
    shared = dict(
        wqt=np.ascontiguousarray(np.vstack([np.asarray(Wq, np.float32).T] * 2)).astype(BF),
        wkt=np.ascontiguousarray(np.vstack([np.asarray(Wk, np.float32).T] * 2)).astype(BF),
        wvt=np.ascontiguousarray(np.vstack([np.asarray(Wv, np.float32).T] * 2)).astype(BF),
        maskq=maskq.astype(BF), e2big=e2big.astype(BF),
        bq2=np.tile(np.asarray(bq, np.float32), 2).reshape(128, 1).copy(),
        bv2=np.tile(bvf, 2).reshape(128, 1).copy(),
        gamma2=np.tile(np.asarray(gamma, np.float32), 2).reshape(128, 1).copy(),
        beta2=np.tile(np.asarray(beta, np.float32), 2).reshape(128, 1).copy(),
    )

    xbf = x.astype(BF)
    in_maps = []
    lays = []
    for d in range(NCORES):
        xin = np.zeros((128, W), BF)
        pfl = np.zeros((2, W), np.float32)
        fc = np.zeros((128, 2), np.float32)
        lay = []
        for h in range(2):
            s = 2 * d + h
            slots, pts, padflag, nfake = _strip_layout(prep, s)
            xin[64 * h: 64 * h + 64, slots] = xbf[pts].T
            pfl[h] = padflag
            bvh = np.tile(bvf, 2).reshape(128)[64 * h: 64 * h + 64]
            fc[64 * h: 64 * h + 64, 0] = nfake * bvh
            fc[64 * h: 64 * h + 64, 1] = nfake * bvh * bvh
            lay.append((slots, pts))
        m = dict(shared)
        m["xin"] = xin
        m["pflag"] = pfl.astype(BF)
        m["fakecorr"] = fc
        in_maps.append(m)
        lays.append(lay)

    return nc, in_maps, lays


def _finish(results, lays):
    out = np.empty((N_TOT, D), np.float32)
    for d in range(NCORES):
        h = results[d]["hout"]
        for si in range(2):
            slots, pts = lays[d][si]
            out[pts] = h[si * 64:(si + 1) * 64, slots].T
    return out


def kernel(**inputs):
    nc, in_maps, lays = _prepare(**inputs)
    res = run_bass_kernel_spmd(nc, in_maps, core_ids=list(range(NCORES)),
                               **getattr(kernel, "run_kwargs", {}))
    kernel.last_results = res
    return _finish(res.results, lays)
